# revision 24
# baseline (speedup 1.0000x reference)
"""Trainium2 Bass kernel for a dense self-attention block (B=4, N=S=1024,
C=768, H=12) with an additive attention-weight bias:

    q = heads(x @ Wq.T); k = heads(x @ Wk.T); v = heads(x @ Wv.T)
    attn = softmax(attn_weight + log_softmax(scale * q k^T))
    out  = (attn @ v) @ Wo.T + bo

Math restructure vs the naive form:
  softmax(w + log_softmax(a)) == softmax(w + a),  exp(w + a) == exp(w)*exp(a)
so the host ships ew = exp(attn_weight) (fp16) and the device computes
E = exp(s) * ew with exp on ScalarE and the multiply on VectorE in fp16
(2 elem/cycle/lane).  This removes the identity-matmul wt-add (~41us of
PE streaming) that dominated the previous version.

Sharding: 8 cores = 4 batches x 2 head-groups (6 heads each).  Host sums
the two head-group partial projections + bias.

Per-core pipeline (three head-pairs):
  qk^T for a head pair is issued as two row-tiled matmuls (array rows
  0:64 / 64:128 via auto tile_position) writing adjacent PSUM banks, so
  the PE can stream both heads concurrently.  ACT exp paces the kernel
  (~50us); PE filler work (QKV projections, PV of the previous pair, the
  output projection) is interleaved between S^T tiles by emission order.
  PV keeps the [v | ones] augmented stationary so the softmax denominator
  rides along; 1/r is reshaped to [128,4] via a small DRAM round trip so
  the multi-pass DVE reciprocal stays 128-way parallel.
All DMA issue stays on the sync/gpsimd queues (ACT + DVE queues are the
busy engines and must not head-of-line block on DMA semaphores).
"""

import ml_dtypes
import numpy as np

BF16 = ml_dtypes.bfloat16

B, N, C, H = 4, 1024, 768, 12
HG = 2                 # head-groups; cores = B*HG = 8
HPG = H // HG          # heads per group = 6
NPAIR = HPG // 2       # head pairs per core = 3
D = C // H             # 64
GJ = HPG * D           # 384
P = 128
SC = N // P            # 8 s-chunks
KC = C // P            # 6 contraction chunks
NCORES = B * HG
SCALE = D ** -0.5
EW_TILES = NPAIR * SC  # 24 ew tiles of [128, 2048]



def build_program():
    import concourse.bass as bass  # noqa: F401
    import concourse.mybir as mybir
    import concourse.tile as tile
    from concourse import bacc

    nc = bacc.Bacc(
        "TRN2",
        target_bir_lowering=False,
        debug=False,
        num_devices=NCORES,
    )
    f32 = mybir.dt.float32
    f32r = mybir.dt.float32r
    f16 = mybir.dt.float16
    bf16 = mybir.dt.bfloat16
    EXP = mybir.ActivationFunctionType.Exp

    # all inputs pre-swizzled on host to the SBUF [partition, ...] layout
    # so every load is one fully-contiguous DMA (4-12KB per partition line)
    xT_d = nc.dram_tensor("xT", [P, KC, N], bf16, kind="ExternalInput").ap()
    wqT_d = nc.dram_tensor("wqT", [P, KC, GJ], bf16,
                           kind="ExternalInput").ap()
    wkT_d = nc.dram_tensor("wkT", [P, KC, GJ], bf16,
                           kind="ExternalInput").ap()
    wvT_d = nc.dram_tensor("wvT", [P, KC, GJ], bf16,
                           kind="ExternalInput").ap()
    woT_d = nc.dram_tensor("woT", [P, NPAIR, C], f16,
                           kind="ExternalInput").ap()
    ew_d = nc.dram_tensor("ew", [EW_TILES, P, 2 * N], f16,
                          kind="ExternalInput").ap()
    out_d = nc.dram_tensor("out", [N, C], f16, kind="ExternalOutput").ap()

    def mm(out, lhsT, rhs, start, stop):
        nc.tensor.matmul(out, lhsT, rhs, start=start, stop=stop)

    with tile.TileContext(nc) as tc:
        with (
            tc.tile_pool(name="const", bufs=1) as const_pool,
            tc.tile_pool(name="Epool", bufs=18) as big_pool,
            tc.tile_pool(name="ewt", bufs=4) as ew_pool,
            tc.tile_pool(name="est", bufs=4) as es_pool,
            tc.tile_pool(name="rtile", bufs=3) as r_pool,
            tc.tile_pool(name="rbtile", bufs=2) as rb_pool,
            tc.tile_pool(name="outtile", bufs=2) as out_pool,
            tc.tile_pool(name="ps_st", bufs=2, space="PSUM") as psum_st,
            tc.tile_pool(name="ps_qkv", bufs=2, space="PSUM") as psum_qkv,
            tc.tile_pool(name="ps_pv", bufs=2, space="PSUM") as psum_pv,
            tc.tile_pool(name="dram", bufs=4, space="DRAM") as dram_pool,
        ):
            # ---- DMA queue round-robin (sync + gpsimd only) ------------
            _dq = [nc.sync, nc.gpsimd]
            _dqi = [0]

            def dq():
                e = _dq[_dqi[0] % 2]
                _dqi[0] += 1
                return e

            # ---- constant tiles ---------------------------------------
            wq_sb = const_pool.tile([P, KC, GJ], bf16)
            wk_sb = const_pool.tile([P, KC, GJ], bf16)
            wv_sb = const_pool.tile([P, KC, GJ], bf16)
            xT_sb = const_pool.tile([P, KC, N], bf16)
            woT_sb = const_pool.tile([P, NPAIR, C], f16)
            qT_sbs = [const_pool.tile([P, N], f32r, name=f"qT{j}")
                      for j in range(NPAIR)]
            kT_sbs = [const_pool.tile([P, N], f32r, name=f"kT{j}")
                      for j in range(NPAIR)]
            oT_sbs = [const_pool.tile([P, N], f16, name=f"oT{j}")
                      for j in range(NPAIR)]
            v_aug = const_pool.tile([P, SC, HPG, P], f16)
            scratch = const_pool.tile([P, 16], f32)

            # single big contiguous transfers (amortize per-DMA fixed cost);
            # nothing else rides HBM until x lands (fill is DMA-critical)
            nc.sync.dma_start(wq_sb, wqT_d)
            nc.gpsimd.dma_start(wk_sb, wkT_d)
            nc.sync.dma_start(xT_sb, xT_d)

            # dummy exp: pulls ACT_TABLE_LOAD off the critical path
            nc.gpsimd.memset(scratch, 1.0)
            nc.scalar.activation(scratch, scratch, EXP)

            # ew tile DMA issue (lookahead-managed)
            ew_tiles: dict[int, object] = {}

            def issue_ew(t):
                if t < EW_TILES and t not in ew_tiles:
                    et = ew_pool.tile([P, 2 * N], f16, tag="ew")
                    (nc.sync if t % 2 == 0 else nc.gpsimd).dma_start(
                        et, ew_d[t])
                    ew_tiles[t] = et

            issue_ew(0)
            issue_ew(1)
            issue_ew(2)
            issue_ew(3)
            nc.gpsimd.dma_start(wv_sb, wvT_d)

            # v_aug pads via memset (no DMA): [v|1] even, [0|1|0|v] odd
            for h in range(HPG):
                if h % 2 == 0:
                    nc.gpsimd.memset(v_aug[:, :, h, 64:65], 1.0)
                else:
                    nc.gpsimd.memset(v_aug[:, :, h, 0:32], 0.0)
                    nc.gpsimd.memset(v_aug[:, :, h, 32:33], 1.0)
                    nc.gpsimd.memset(v_aug[:, :, h, 33:64], 0.0)

            # deferred-work scheduler: run closure N "ticks" (ST tiles)
            # after scheduling, so cross-engine chains never head-of-line
            # block the DVE queue.
            pending: list = []

            def after(ticks, fn):
                pending.append([ticks, fn])

            def tick():
                ready = []
                for ev in pending:
                    ev[0] -= 1
                    if ev[0] <= 0:
                        ready.append(ev)
                for ev in ready:
                    pending.remove(ev)
                    ev[1]()

            def flush():
                # run in ascending remaining-tick order so short chains
                # (reciprocal stages) never sit behind blocking muls
                for ev in sorted(pending, key=lambda e: e[0]):
                    ev[1]()
                pending.clear()

            # ---- emission helpers -------------------------------------
            def emit_qk_half(m, which, nb):
                ps = psum_qkv.tile([P, 512], f32, tag="qkv")
                wsb = wq_sb if which == "q" else wk_sb
                dst = qT_sbs[m] if which == "q" else kT_sbs[m]
                ncol = slice(nb * 512, (nb + 1) * 512)
                for kc in range(KC):
                    mm(ps, wsb[:, kc, m * P:(m + 1) * P],
                       xT_sb[:, kc, ncol],
                       start=(kc == 0), stop=(kc == KC - 1))
                nc.vector.tensor_copy(dst[:, ncol], ps)

            def emit_v_sc(sc):
                ps = psum_qkv.tile([P, 512], f32, tag="qkv")
                for kc in range(KC):
                    mm(ps[:, :GJ], xT_sb[:, kc, sc * P:(sc + 1) * P],
                       wv_sb[:, kc, :], start=(kc == 0), stop=(kc == KC - 1))
                vsrc = ps[:, :GJ].rearrange("p (h d) -> p h d", d=D)
                nc.scalar.copy(v_aug[:, sc, 0:HPG:2, 0:64],
                               vsrc[:, 0:HPG:2, :])
                nc.scalar.copy(v_aug[:, sc, 1:HPG:2, 64:128],
                               vsrc[:, 1:HPG:2, :])

            es_tiles: dict[int, object] = {}
            E_tiles: dict[tuple, object] = {}

            def emit_st_tile(pair, sc, nb):
                key = pair * SC + sc
                if key not in es_tiles:
                    es_tiles[key] = es_pool.tile([P, 2 * N], f16, tag="es",
                                                 name=f"es{key}")
                es_t = es_tiles[key]
                ps = psum_st.tile([P, N], f32, tag="st")
                scol = slice(sc * P, (sc + 1) * P)
                ncol = slice(nb * 512, (nb + 1) * 512)
                mm(ps[:, 0:512], kT_sbs[pair][0:64, scol],
                   qT_sbs[pair][0:64, ncol], start=True, stop=True)
                mm(ps[:, 512:1024], kT_sbs[pair][64:128, scol],
                   qT_sbs[pair][64:128, ncol], start=True, stop=True)
                nc.scalar.activation(es_t[:, nb * N:(nb + 1) * N], ps, EXP)

            def emit_mult(pair, sc):
                key = pair * SC + sc
                E_t = big_pool.tile([P, 2 * N], f16, tag="E")
                nc.vector.tensor_mul(E_t, es_tiles.pop(key),
                                     ew_tiles.pop(key))
                E_tiles[(pair, sc)] = E_t

            def emit_pv_pair(pair, nb, alt_pool=False, tail_q=None):
                # both heads of the pair: hh=0 -> [v|1] stationary, psum
                # rows 0:65 (r at 64); hh=1 -> [0|1|0|v], rows 32 + 64:128
                psos = []
                for hh in range(2):
                    h = 2 * pair + hh
                    pool = psum_qkv if alt_pool else psum_pv
                    pso = pool.tile([P, 512], f32,
                                    tag=("qkv" if alt_pool else "pv"))
                    for sc in range(SC):
                        lh = (v_aug[:, sc, h, 0:65] if hh == 0
                              else v_aug[:, sc, h, 0:P])
                        po = (pso[0:65, :] if hh == 0 else pso[:, :])
                        mm(po, lh,
                           E_tiles[(pair, sc)][:, nb * N + hh * 512:
                                               nb * N + hh * 512 + 512],
                           start=(sc == 0), stop=(sc == SC - 1))
                    psos.append(pso)
                if nb == 1:
                    for sc in range(SC):
                        E_tiles.pop((pair, sc))
                # batched r-normalization for both heads: rows at psum
                # partitions 64 (hh0) and 32 (hh1); one DRAM round trip
                # reshapes to [128,8] so the multi-pass DVE reciprocal is
                # lane-parallel, then broadcast back along partitions.
                d = tail_q if tail_q is not None else nc.sync
                rb = rb_pool.tile([P, 512], f32, tag="rb")
                r_t = r_pool.tile([P, 512], f32, tag="r")
                nc.vector.tensor_copy(r_t[64:65, :], psos[0][64:65, :])
                nc.vector.tensor_copy(r_t[32:33, :], psos[1][32:33, :])
                rd1 = dram_pool.tile([2, 512], f32, tag="rd1")
                d.dma_start(rd1[0:1, :], r_t[64:65, :])
                d.dma_start(rd1[1:2, :], r_t[32:33, :])
                rsq = r_pool.tile([P, 8], f32, tag="rsq")
                d.dma_start(rsq, rd1.rearrange("a (p o) -> (a p) o", p=64))

                def recip_stage(d=d, rsq=rsq, rd1=rd1, rb=rb):
                    rd2 = dram_pool.tile([2, 512], f32, tag="rd2")
                    nc.vector.reciprocal(rsq, rsq)
                    d.dma_start(
                        rd2.rearrange("a (p o) -> (a p) o", p=64), rsq)
                    d.dma_start(rb[0:64, :],
                                rd2[0:1, :].partition_broadcast(64))
                    d.dma_start(rb[64:128, :],
                                rd2[1:2, :].partition_broadcast(64))
                after(3, recip_stage)

                def mul(psos=psos, rb=rb):
                    ncol = slice(nb * 512, (nb + 1) * 512)
                    nc.vector.tensor_mul(oT_sbs[pair][0:64, ncol],
                                         psos[0][0:64, :], rb[0:64, :])
                    nc.vector.tensor_mul(oT_sbs[pair][64:128, ncol],
                                         psos[1][64:128, :], rb[64:128, :])
                after(6, mul)

            def emit_proj(nbk):
                # even chunks use the ps_st ring, odd chunks borrow the
                # (tail-idle) pv+qkv pools -> 4-deep psum pipeline
                if nbk % 2 == 0:
                    ps = psum_st.tile([P, N], f32, tag="st")
                    parts = [(ps[:, 0:512], slice(0, 512)),
                             (ps[:, 512:C], slice(512, C))]
                else:
                    psa = psum_pv.tile([P, 512], f32, tag="pv")
                    psb = psum_qkv.tile([P, 512], f32, tag="qkv")
                    parts = [(psa[:, :], slice(0, 512)),
                             (psb[:, 0:C - 512], slice(512, C))]
                for ptile, ccol in parts:
                    for j3 in range(NPAIR):
                        mm(ptile,
                           oT_sbs[j3][:, nbk * P:(nbk + 1) * P],
                           woT_sb[:, j3, ccol],
                           start=(j3 == 0), stop=(j3 == NPAIR - 1))
                ob = out_pool.tile([P, C], f16, tag="ob")
                for ptile, ccol in parts:
                    nc.scalar.copy(ob[:, ccol], ptile)
                nc.gpsimd.dma_start(
                    out_d.rearrange("(o p) c -> o p c", p=P)[nbk], ob)

            # ---- fill: q/k for pair 0 (both halves) --------------------
            emit_qk_half(0, "q", 0)
            emit_qk_half(0, "k", 0)
            emit_qk_half(0, "q", 1)
            emit_qk_half(0, "k", 1)

            # ---- main pipeline ----------------------------------------
            # ST tile order per pair: sc0/sc1 nb0 first so the first two
            # ACT exps depend only on the nb0 halves computed in the fill.
            def st_order():
                yield 0, (0, 0)
                yield 1, (1, 0)
                yield 2, (0, 1)
                yield 3, (1, 1)
                i = 4
                for sc in range(2, SC):
                    yield i, (sc, 0)
                    yield i + 1, (sc, 1)
                    i += 2

            fillers = {
                0: [lambda: emit_qk_half(1, "q", 0),
                    lambda: emit_qk_half(1, "k", 0),
                    lambda: emit_qk_half(1, "q", 1),
                    lambda: emit_qk_half(1, "k", 1)]
                   + [(lambda s: lambda: emit_v_sc(s))(s) for s in range(SC)],
                1: [lambda: emit_qk_half(2, "q", 0),
                    lambda: emit_qk_half(2, "k", 0),
                    lambda: emit_qk_half(2, "q", 1),
                    lambda: emit_qk_half(2, "k", 1),
                    lambda: emit_pv_pair(0, 0),
                    lambda: emit_pv_pair(0, 1, alt_pool=True)],
                2: [lambda: emit_pv_pair(1, 0),
                    lambda: emit_pv_pair(1, 1, alt_pool=True)],
            }

            for pair in range(NPAIR):
                fl = list(fillers[pair])
                if pair == 1:
                    # late-load woT (needed only at the tail)
                    nc.sync.dma_start(woT_sb, woT_d)
                for i, (sc, nb) in st_order():
                    emit_st_tile(pair, sc, nb)
                    issue_ew(pair * SC + sc + 2)
                    if i in (2, 3):
                        emit_mult(pair, i - 2)
                    elif i >= 5 and i % 2 == 1:
                        emit_mult(pair, (i - 1) // 2)
                    if fl:
                        fl.pop(0)()
                    tick()
                for f in fl:
                    f()
                    tick()

            # ---- tail: PV pair 2 (nb-major, dense) + projection --------
            emit_pv_pair(2, 0, tail_q=nc.scalar)
            emit_pv_pair(2, 1, alt_pool=True, tail_q=nc.sync)
            flush()
            for nbk in range(SC):
                emit_proj(nbk)

    nc.compile()
    return nc


_PROG = None


def _get_prog():
    global _PROG
    if _PROG is None:
        _PROG = build_program()
    return _PROG


def make_in_maps(query, attn_weight, Wq, Wk, Wv, Wo):
    query = np.asarray(query, dtype=np.float32)
    attn_weight = np.asarray(attn_weight, dtype=np.float32)
    Wq = np.asarray(Wq, dtype=np.float32)
    Wk = np.asarray(Wk, dtype=np.float32)
    Wv = np.asarray(Wv, dtype=np.float32)
    Wo = np.asarray(Wo, dtype=np.float32)

    def swiz(a):
        # [C_or_GJ, cols] -> [P, o, cols] partition-major (SBUF layout)
        o = a.shape[0] // P
        return np.ascontiguousarray(a.reshape(o, P, -1).transpose(1, 0, 2))

    in_maps = []
    for b in range(B):
        xT = swiz(np.ascontiguousarray(query[b].T).astype(BF16))
        for g in range(HG):
            rows = slice(g * GJ, (g + 1) * GJ)
            wqT = swiz(((SCALE * Wq[rows, :]).T).astype(BF16))
            wkT = swiz((Wk[rows, :].T).astype(BF16))
            wvT = swiz((Wv[rows, :].T).astype(BF16))
            woT = swiz((Wo[:, rows].T).astype(np.float16))
            # ew tiles: [pair, sc, p(s), (nb, hh, j(n))] -> [24, 128, 2048]
            w6 = attn_weight[b, g * HPG:(g + 1) * HPG]
            ewT = np.exp(w6, dtype=np.float32).transpose(0, 2, 1)
            e6 = np.ascontiguousarray(ewT).reshape(NPAIR, 2, SC, P, 2, 512)
            ew = np.ascontiguousarray(
                e6.transpose(0, 2, 3, 4, 1, 5)
            ).reshape(EW_TILES, P, 2 * N).astype(np.float16)
            in_maps.append({
                "xT": xT, "wqT": wqT, "wkT": wkT, "wvT": wvT,
                "woT": woT, "ew": ew,
            })
    return in_maps


def run(inputs, trace=False, **spmd_kwargs):
    """Execute on 8 cores; returns (full_output, BassKernelResults)."""
    from concourse import bass_utils

    nc = _get_prog()
    in_maps = make_in_maps(inputs["query"], inputs["attn_weight"],
                           inputs["Wq"], inputs["Wk"], inputs["Wv"],
                           inputs["Wo"])
    res = bass_utils.run_bass_kernel_spmd(
        nc, in_maps, core_ids=list(range(NCORES)), trace=trace, **spmd_kwargs)
    bo = np.asarray(inputs["bo"], dtype=np.float32)
    full = np.empty((B, N, C), dtype=np.float32)
    for b in range(B):
        full[b] = (res.results[2 * b]["out"].astype(np.float32)
                   + res.results[2 * b + 1]["out"].astype(np.float32) + bo)
    return full, res


def kernel(**inputs):
    full, _ = run(inputs, trace=False)
    return full


# revision 25
# speedup vs baseline: 1.0975x; 1.0975x over previous
"""Trainium2 Bass kernel for a dense self-attention block (B=4, N=S=1024,
C=768, H=12) with an additive attention-weight bias:

    q = heads(x @ Wq.T); k = heads(x @ Wk.T); v = heads(x @ Wv.T)
    attn = softmax(attn_weight + log_softmax(scale * q k^T))
    out  = (attn @ v) @ Wo.T + bo

Math restructure vs the naive form:
  softmax(w + log_softmax(a)) == softmax(w + a),  exp(w + a) == exp(w)*exp(a)
so the host ships ew = exp(attn_weight) (fp16) and the device computes
E = exp(s) * ew with exp on ScalarE and the multiply on VectorE in fp16
(2 elem/cycle/lane).  This removes the identity-matmul wt-add (~41us of
PE streaming) that dominated the previous version.

Sharding: 8 cores = 4 batches x 2 head-groups (6 heads each).  Host sums
the two head-group partial projections + bias.

Per-core pipeline (three head-pairs):
  qk^T for a head pair is issued as two row-tiled matmuls (array rows
  0:64 / 64:128 via auto tile_position) writing adjacent PSUM banks, so
  the PE can stream both heads concurrently.  ACT exp paces the kernel
  (~50us); PE filler work (QKV projections, PV of the previous pair, the
  output projection) is interleaved between S^T tiles by emission order.
  PV keeps the [v | ones] augmented stationary so the softmax denominator
  rides along; 1/r is reshaped to [128,4] via a small DRAM round trip so
  the multi-pass DVE reciprocal stays 128-way parallel.
All DMA issue stays on the sync/gpsimd queues (ACT + DVE queues are the
busy engines and must not head-of-line block on DMA semaphores).
"""

import ml_dtypes
import numpy as np

BF16 = ml_dtypes.bfloat16

B, N, C, H = 4, 1024, 768, 12
HG = 2                 # head-groups; cores = B*HG = 8
HPG = H // HG          # heads per group = 6
NPAIR = HPG // 2       # head pairs per core = 3
D = C // H             # 64
GJ = HPG * D           # 384
P = 128
SC = N // P            # 8 s-chunks
KC = C // P            # 6 contraction chunks
NCORES = B * HG
SCALE = D ** -0.5
EW_TILES = NPAIR * SC  # 24 ew tiles of [128, 2048]



def build_program():
    import concourse.bass as bass  # noqa: F401
    import concourse.mybir as mybir
    import concourse.tile as tile
    from concourse import bacc

    nc = bacc.Bacc(
        "TRN2",
        target_bir_lowering=False,
        debug=False,
        num_devices=NCORES,
    )
    f32 = mybir.dt.float32
    f32r = mybir.dt.float32r
    f16 = mybir.dt.float16
    bf16 = mybir.dt.bfloat16
    EXP = mybir.ActivationFunctionType.Exp

    # all inputs pre-swizzled on host to the SBUF [partition, ...] layout
    # so every load is one fully-contiguous DMA (4-12KB per partition line)
    xT_d = nc.dram_tensor("xT", [P, KC, N], bf16, kind="ExternalInput").ap()
    wqT_d = nc.dram_tensor("wqT", [P, KC, GJ], bf16,
                           kind="ExternalInput").ap()
    wkT_d = nc.dram_tensor("wkT", [P, KC, GJ], bf16,
                           kind="ExternalInput").ap()
    wvT_d = nc.dram_tensor("wvT", [P, KC, GJ], bf16,
                           kind="ExternalInput").ap()
    woT_d = nc.dram_tensor("woT", [P, NPAIR, C], f16,
                           kind="ExternalInput").ap()
    ew_d = nc.dram_tensor("ew", [EW_TILES, P, 2 * N], f16,
                          kind="ExternalInput").ap()
    out_d = nc.dram_tensor("out", [N, C], f16, kind="ExternalOutput").ap()

    def mm(out, lhsT, rhs, start, stop):
        nc.tensor.matmul(out, lhsT, rhs, start=start, stop=stop)

    with tile.TileContext(nc) as tc:
        with (
            tc.tile_pool(name="const", bufs=1) as const_pool,
            tc.tile_pool(name="Epool", bufs=18) as big_pool,
            tc.tile_pool(name="ewt", bufs=4) as ew_pool,
            tc.tile_pool(name="est", bufs=4) as es_pool,
            tc.tile_pool(name="rtile", bufs=3) as r_pool,
            tc.tile_pool(name="rbtile", bufs=2) as rb_pool,
            tc.tile_pool(name="outtile", bufs=2) as out_pool,
            tc.tile_pool(name="ps_st", bufs=2, space="PSUM") as psum_st,
            tc.tile_pool(name="ps_qkv", bufs=2, space="PSUM") as psum_qkv,
            tc.tile_pool(name="ps_pv", bufs=2, space="PSUM") as psum_pv,
            tc.tile_pool(name="dram", bufs=4, space="DRAM") as dram_pool,
        ):
            # ---- DMA queue round-robin (sync + gpsimd only) ------------
            _dq = [nc.sync, nc.gpsimd]
            _dqi = [0]

            def dq():
                e = _dq[_dqi[0] % 2]
                _dqi[0] += 1
                return e

            # ---- constant tiles ---------------------------------------
            wq_sb = const_pool.tile([P, KC, GJ], bf16)
            wk_sb = const_pool.tile([P, KC, GJ], bf16)
            wv_sb = const_pool.tile([P, KC, GJ], bf16)
            xT_sb = const_pool.tile([P, KC, N], bf16)
            woT_sb = const_pool.tile([P, NPAIR, C], f16)
            qT_sbs = [const_pool.tile([P, N], f32r, name=f"qT{j}")
                      for j in range(NPAIR)]
            kT_sbs = [const_pool.tile([P, N], f32r, name=f"kT{j}")
                      for j in range(NPAIR)]
            oT_sbs = [const_pool.tile([P, N], f16, name=f"oT{j}")
                      for j in range(NPAIR)]
            v_aug = const_pool.tile([P, SC, HPG, P], f16)
            scratch = const_pool.tile([P, 16], f32)

            # single big contiguous transfers (amortize per-DMA fixed cost);
            # nothing else rides HBM until x lands (fill is DMA-critical)
            # x is the fill-critical transfer: give it the sync ring
            # alone; everything else shares the gpsimd ring behind it
            nc.sync.dma_start(xT_sb, xT_d)
            nc.gpsimd.dma_start(wq_sb, wqT_d)
            nc.gpsimd.dma_start(wk_sb, wkT_d)
            nc.gpsimd.dma_start(wv_sb, wvT_d)

            # dummy exp: pulls ACT_TABLE_LOAD off the critical path
            nc.gpsimd.memset(scratch, 1.0)
            nc.scalar.activation(scratch, scratch, EXP)

            # ew tile DMA issue (lookahead-managed)
            ew_tiles: dict[int, object] = {}

            def issue_ew(t):
                if t < EW_TILES and t not in ew_tiles:
                    et = ew_pool.tile([P, 2 * N], f16, tag="ew")
                    (nc.sync if t % 2 == 0 else nc.gpsimd).dma_start(
                        et, ew_d[t])
                    ew_tiles[t] = et

            issue_ew(0)
            issue_ew(1)
            issue_ew(2)
            issue_ew(3)
            # (even-t tiles ride sync after x; odd-t behind wq/wk/wv)

            # v_aug pads via memset (no DMA): [v|1] even, [0|1|0|v] odd
            for h in range(HPG):
                if h % 2 == 0:
                    nc.gpsimd.memset(v_aug[:, :, h, 64:65], 1.0)
                else:
                    nc.gpsimd.memset(v_aug[:, :, h, 0:32], 0.0)
                    nc.gpsimd.memset(v_aug[:, :, h, 32:33], 1.0)
                    nc.gpsimd.memset(v_aug[:, :, h, 33:64], 0.0)

            # deferred-work scheduler: run closure N "ticks" (ST tiles)
            # after scheduling, so cross-engine chains never head-of-line
            # block the DVE queue.
            pending: list = []

            def after(ticks, fn):
                pending.append([ticks, fn])

            def tick():
                ready = []
                for ev in pending:
                    ev[0] -= 1
                    if ev[0] <= 0:
                        ready.append(ev)
                for ev in ready:
                    pending.remove(ev)
                    ev[1]()

            def flush():
                # run in ascending remaining-tick order so short chains
                # (reciprocal stages) never sit behind blocking muls
                for ev in sorted(pending, key=lambda e: e[0]):
                    ev[1]()
                pending.clear()

            # ---- emission helpers -------------------------------------
            def emit_qk_half(m, which, nb):
                ps = psum_qkv.tile([P, 512], f32, tag="qkv")
                wsb = wq_sb if which == "q" else wk_sb
                dst = qT_sbs[m] if which == "q" else kT_sbs[m]
                ncol = slice(nb * 512, (nb + 1) * 512)
                for kc in range(KC):
                    mm(ps, wsb[:, kc, m * P:(m + 1) * P],
                       xT_sb[:, kc, ncol],
                       start=(kc == 0), stop=(kc == KC - 1))
                nc.vector.tensor_copy(dst[:, ncol], ps)

            def emit_v_sc(sc):
                ps = psum_qkv.tile([P, 512], f32, tag="qkv")
                for kc in range(KC):
                    mm(ps[:, :GJ], xT_sb[:, kc, sc * P:(sc + 1) * P],
                       wv_sb[:, kc, :], start=(kc == 0), stop=(kc == KC - 1))
                vsrc = ps[:, :GJ].rearrange("p (h d) -> p h d", d=D)
                nc.scalar.copy(v_aug[:, sc, 0:HPG:2, 0:64],
                               vsrc[:, 0:HPG:2, :])
                nc.scalar.copy(v_aug[:, sc, 1:HPG:2, 64:128],
                               vsrc[:, 1:HPG:2, :])

            es_tiles: dict[int, object] = {}
            E_tiles: dict[tuple, object] = {}

            def emit_st_tile(pair, sc, nb):
                key = pair * SC + sc
                if key not in es_tiles:
                    es_tiles[key] = es_pool.tile([P, 2 * N], f16, tag="es",
                                                 name=f"es{key}")
                es_t = es_tiles[key]
                ps = psum_st.tile([P, N], f32, tag="st")
                scol = slice(sc * P, (sc + 1) * P)
                ncol = slice(nb * 512, (nb + 1) * 512)
                mm(ps[:, 0:512], kT_sbs[pair][0:64, scol],
                   qT_sbs[pair][0:64, ncol], start=True, stop=True)
                mm(ps[:, 512:1024], kT_sbs[pair][64:128, scol],
                   qT_sbs[pair][64:128, ncol], start=True, stop=True)
                nc.scalar.activation(es_t[:, nb * N:(nb + 1) * N], ps, EXP)

            def emit_mult(pair, sc):
                key = pair * SC + sc
                E_t = big_pool.tile([P, 2 * N], f16, tag="E")
                nc.vector.tensor_mul(E_t, es_tiles.pop(key),
                                     ew_tiles.pop(key))
                E_tiles[(pair, sc)] = E_t

            def emit_pv_pair(pair, nb, alt_pool=False, tail_q=None):
                # both heads of the pair: hh=0 -> [v|1] stationary, psum
                # rows 0:65 (r at 64); hh=1 -> [0|1|0|v], rows 32 + 64:128
                psos = []
                for hh in range(2):
                    h = 2 * pair + hh
                    pool = psum_qkv if alt_pool else psum_pv
                    pso = pool.tile([P, 512], f32,
                                    tag=("qkv" if alt_pool else "pv"))
                    for sc in range(SC):
                        lh = (v_aug[:, sc, h, 0:65] if hh == 0
                              else v_aug[:, sc, h, 0:P])
                        po = (pso[0:65, :] if hh == 0 else pso[:, :])
                        mm(po, lh,
                           E_tiles[(pair, sc)][:, nb * N + hh * 512:
                                               nb * N + hh * 512 + 512],
                           start=(sc == 0), stop=(sc == SC - 1))
                    psos.append(pso)
                if nb == 1:
                    for sc in range(SC):
                        E_tiles.pop((pair, sc))
                # batched r-normalization for both heads: rows at psum
                # partitions 64 (hh0) and 32 (hh1); one DRAM round trip
                # reshapes to [128,8] so the multi-pass DVE reciprocal is
                # lane-parallel, then broadcast back along partitions.
                d = tail_q if tail_q is not None else nc.sync
                rb = rb_pool.tile([P, 512], f32, tag="rb")
                r_t = r_pool.tile([P, 512], f32, tag="r")
                nc.vector.tensor_copy(r_t[64:65, :], psos[0][64:65, :])
                nc.vector.tensor_copy(r_t[32:33, :], psos[1][32:33, :])
                rd1 = dram_pool.tile([2, 512], f32, tag="rd1")
                d.dma_start(rd1[0:1, :], r_t[64:65, :])
                d.dma_start(rd1[1:2, :], r_t[32:33, :])
                rsq = r_pool.tile([P, 8], f32, tag="rsq")
                d.dma_start(rsq, rd1.rearrange("a (p o) -> (a p) o", p=64))

                def recip_stage(d=d, rsq=rsq, rd1=rd1, rb=rb):
                    rd2 = dram_pool.tile([2, 512], f32, tag="rd2")
                    nc.vector.reciprocal(rsq, rsq)
                    d.dma_start(
                        rd2.rearrange("a (p o) -> (a p) o", p=64), rsq)
                    d.dma_start(rb[0:64, :],
                                rd2[0:1, :].partition_broadcast(64))
                    d.dma_start(rb[64:128, :],
                                rd2[1:2, :].partition_broadcast(64))
                after(3, recip_stage)

                def mul(psos=psos, rb=rb):
                    ncol = slice(nb * 512, (nb + 1) * 512)
                    nc.vector.tensor_mul(oT_sbs[pair][0:64, ncol],
                                         psos[0][0:64, :], rb[0:64, :])
                    nc.vector.tensor_mul(oT_sbs[pair][64:128, ncol],
                                         psos[1][64:128, :], rb[64:128, :])
                after(6, mul)

            def emit_proj(nbk):
                ps = psum_st.tile([P, N], f32, tag="st")
                for cb in range(2):
                    cw = 512 if cb == 0 else C - 512
                    ccol = slice(cb * 512, cb * 512 + cw)
                    for j3 in range(NPAIR):
                        mm(ps[:, ccol],
                           oT_sbs[j3][:, nbk * P:(nbk + 1) * P],
                           woT_sb[:, j3, ccol],
                           start=(j3 == 0), stop=(j3 == NPAIR - 1))
                ob = out_pool.tile([P, C], f16, tag="ob")
                nc.scalar.copy(ob, ps[:, :C])
                nc.sync.dma_start(
                    out_d.rearrange("(o p) c -> o p c", p=P)[nbk], ob)

            # ---- fill: q/k for pair 0 (both halves) --------------------
            emit_qk_half(0, "q", 0)
            emit_qk_half(0, "k", 0)
            emit_qk_half(0, "q", 1)
            emit_qk_half(0, "k", 1)

            # ---- main pipeline ----------------------------------------
            # ST tile order per pair: sc0/sc1 nb0 first so the first two
            # ACT exps depend only on the nb0 halves computed in the fill.
            def st_order():
                yield 0, (0, 0)
                yield 1, (1, 0)
                yield 2, (0, 1)
                yield 3, (1, 1)
                i = 4
                for sc in range(2, SC):
                    yield i, (sc, 0)
                    yield i + 1, (sc, 1)
                    i += 2

            fillers = {
                0: [lambda: emit_qk_half(1, "q", 0),
                    lambda: emit_qk_half(1, "k", 0),
                    lambda: emit_qk_half(1, "q", 1),
                    lambda: emit_qk_half(1, "k", 1)]
                   + [(lambda s: lambda: emit_v_sc(s))(s) for s in range(SC)],
                1: [lambda: emit_qk_half(2, "q", 0),
                    lambda: emit_qk_half(2, "k", 0),
                    lambda: emit_qk_half(2, "q", 1),
                    lambda: emit_qk_half(2, "k", 1),
                    lambda: emit_pv_pair(0, 0),
                    lambda: emit_pv_pair(0, 1, alt_pool=True)],
                2: [lambda: emit_pv_pair(1, 0),
                    lambda: emit_pv_pair(1, 1, alt_pool=True)],
            }

            for pair in range(NPAIR):
                fl = list(fillers[pair])
                if pair == 1:
                    # late-load woT (needed only at the tail)
                    nc.sync.dma_start(woT_sb, woT_d)
                for i, (sc, nb) in st_order():
                    emit_st_tile(pair, sc, nb)
                    issue_ew(pair * SC + sc + 2)
                    if i in (2, 3):
                        emit_mult(pair, i - 2)
                    elif i >= 5 and i % 2 == 1:
                        emit_mult(pair, (i - 1) // 2)
                    if fl:
                        fl.pop(0)()
                    tick()
                for f in fl:
                    f()
                    tick()

            # ---- tail: PV pair 2 (nb-major, dense) + projection --------
            emit_pv_pair(2, 0, tail_q=nc.scalar)
            emit_pv_pair(2, 1, alt_pool=True, tail_q=nc.sync)
            flush()
            for nbk in range(SC):
                emit_proj(nbk)

    nc.compile()
    return nc


_PROG = None


def _get_prog():
    global _PROG
    if _PROG is None:
        _PROG = build_program()
    return _PROG


def make_in_maps(query, attn_weight, Wq, Wk, Wv, Wo):
    query = np.asarray(query, dtype=np.float32)
    attn_weight = np.asarray(attn_weight, dtype=np.float32)
    Wq = np.asarray(Wq, dtype=np.float32)
    Wk = np.asarray(Wk, dtype=np.float32)
    Wv = np.asarray(Wv, dtype=np.float32)
    Wo = np.asarray(Wo, dtype=np.float32)

    def swiz(a):
        # [C_or_GJ, cols] -> [P, o, cols] partition-major (SBUF layout)
        o = a.shape[0] // P
        return np.ascontiguousarray(a.reshape(o, P, -1).transpose(1, 0, 2))

    in_maps = []
    for b in range(B):
        xT = swiz(np.ascontiguousarray(query[b].T).astype(BF16))
        for g in range(HG):
            rows = slice(g * GJ, (g + 1) * GJ)
            wqT = swiz(((SCALE * Wq[rows, :]).T).astype(BF16))
            wkT = swiz((Wk[rows, :].T).astype(BF16))
            wvT = swiz((Wv[rows, :].T).astype(BF16))
            woT = swiz((Wo[:, rows].T).astype(np.float16))
            # ew tiles: [pair, sc, p(s), (nb, hh, j(n))] -> [24, 128, 2048]
            w6 = attn_weight[b, g * HPG:(g + 1) * HPG]
            ewT = np.exp(w6, dtype=np.float32).transpose(0, 2, 1)
            e6 = np.ascontiguousarray(ewT).reshape(NPAIR, 2, SC, P, 2, 512)
            ew = np.ascontiguousarray(
                e6.transpose(0, 2, 3, 4, 1, 5)
            ).reshape(EW_TILES, P, 2 * N).astype(np.float16)
            in_maps.append({
                "xT": xT, "wqT": wqT, "wkT": wkT, "wvT": wvT,
                "woT": woT, "ew": ew,
            })
    return in_maps


def run(inputs, trace=False, **spmd_kwargs):
    """Execute on 8 cores; returns (full_output, BassKernelResults)."""
    from concourse import bass_utils

    nc = _get_prog()
    in_maps = make_in_maps(inputs["query"], inputs["attn_weight"],
                           inputs["Wq"], inputs["Wk"], inputs["Wv"],
                           inputs["Wo"])
    res = bass_utils.run_bass_kernel_spmd(
        nc, in_maps, core_ids=list(range(NCORES)), trace=trace, **spmd_kwargs)
    bo = np.asarray(inputs["bo"], dtype=np.float32)
    full = np.empty((B, N, C), dtype=np.float32)
    for b in range(B):
        full[b] = (res.results[2 * b]["out"].astype(np.float32)
                   + res.results[2 * b + 1]["out"].astype(np.float32) + bo)
    return full, res


def kernel(**inputs):
    full, _ = run(inputs, trace=False)
    return full
